# revision 1
# baseline (speedup 1.0000x reference)
"""Trainium2 Bass kernel for nn_Conjunction_57793079935283.

Math: the reference expands weights via ROW_IDX = tile(arange(16), 32)
(only weight rows 0..15 are used; feature i belongs to group g = i//16
with weight row r = i%16).  The whole computation collapses to

  m[b,r]  = max_g |x[b, 16g+r]|
  s[b,r]  = sum_g |x[b, 16g+r]|
  A[b,r]  = sum_g relu(x[b,16g+r] + 1)
  B'[b,r] = sum_g sign(x[b,16g+r] + 1)        (mask count B = (B'+G)/2)

  out = A@w16 - 0.5*B'@w16 - (G/2)*colsum(w16)   ( == (x*mask) @ W )
        - 0.1 * (s @ |w16|)                      ( == -0.1*sum-part )
        + max_r m[b,r] * (0.1*|w16[r,o]|)        ( == +0.1*max-part )

Sharding: tensor-parallel over out_features (8 cores x 128 columns).
Per core the max-part multiply m[b,r]*0.1|w[r,o]| runs on the Tensor
engine as a matmul against a block-diagonal rhs D (host-precomputed
from weights, bf16), landing tmp[b,(r,o)] in PSUM; strided reduce_max
over r gives the max-part.  x is DMAed in two column halves so the
half-1 reductions overlap the half-2 transfer.
"""

import numpy as np

_PROG = None

B = 128          # batch
G = 32           # groups per feature row
R = 16           # weight rows used (multiplicity)
OUT = 1024       # out features
NCORES = 8
OC = OUT // NCORES  # out cols per core (128)
H = G * R // 2      # 256, one column half of x


def _build_program():
    import concourse.bacc as bacc
    import concourse.mybir as mybir
    import concourse.tile as tile
    from concourse import masks

    nc = bacc.Bacc(
        "TRN2", target_bir_lowering=False, debug=False, enable_asserts=False
    )
    f32 = mybir.dt.float32
    bf16 = mybir.dt.bfloat16
    AX = mybir.AxisListType
    Alu = mybir.AluOpType
    Act = mybir.ActivationFunctionType

    x_d = nc.dram_tensor("x", [B, G * R], f32, kind="ExternalInput")
    d_d = nc.dram_tensor("d", [R, R * OC], bf16, kind="ExternalInput")
    rhs_d = nc.dram_tensor("rhs", [3 * R + 1, OC], f32, kind="ExternalInput")
    out_d = nc.dram_tensor("out", [B, OC], f32, kind="ExternalOutput")

    with tile.TileContext(nc) as tc:
        with (
            tc.tile_pool(name="sb", bufs=1) as sb,
            tc.tile_pool(name="ps", bufs=1, space="PSUM") as ps,
        ):
            x = sb.tile([B, G * R], f32)
            d = sb.tile([R, R * OC], bf16)
            rhs = sb.tile([3 * R + 1, OC], f32)
            ident = sb.tile([B, B], f32)
            dummy = sb.tile([B, 8], f32)

            # x half 1 alone on the SP queue (arrives first); half 2 on the
            # Activation queue; weights queued behind half 1
            nc.sync.dma_start(x[:, 0:H], x_d[:, 0:H])
            nc.scalar.dma_start(x[:, H : 2 * H], x_d[:, H : 2 * H])
            nc.sync.dma_start(d[:], d_d[:])
            nc.sync.dma_start(rhs[:], rhs_d[:])

            # GpSimd prep while DMAs fly
            dsrc = sb.tile([B, 8], f32)
            nc.gpsimd.memset(dsrc[:], 0.0)
            masks.make_identity(nc, ident[:])

            # ScalarE: force the ACT table load now, off the critical path
            nc.scalar.activation(dummy[:], dsrc[:], Act.Relu, bias=1.0)

            m1 = sb.tile([B, R], f32)
            m2 = sb.tile([B, R], f32)
            m = sb.tile([B, R], f32)
            s1 = sb.tile([B, R], f32)
            s2 = sb.tile([B, R], f32)
            stack3 = sb.tile([B, 3 * R + 1], f32)
            lhsT = sb.tile([3 * R + 1, B], f32)
            mT = sb.tile([R, B], bf16)

            def half_view(t, h):
                return t[:, h * H : (h + 1) * H].rearrange(
                    "p (g r) -> p r g", g=G // 2, r=R
                )

            # ScalarE per half: relu(x+1), sign(x+1) packed into one tile
            rs1 = sb.tile([B, 2 * H], f32)
            rs2 = sb.tile([B, 2 * H], f32)
            nc.scalar.activation(rs1[:, 0:H], x[:, 0:H], Act.Relu, bias=1.0)
            nc.scalar.activation(rs1[:, H : 2 * H], x[:, 0:H], Act.Sign, bias=1.0)
            nc.scalar.activation(rs2[:, 0:H], x[:, H : 2 * H], Act.Relu, bias=1.0)
            nc.scalar.activation(
                rs2[:, H : 2 * H], x[:, H : 2 * H], Act.Sign, bias=1.0
            )

            # DVE: m/s half-reductions; half 1 runs during half-2 DMA
            nc.vector.tensor_reduce(
                m1[:], half_view(x, 0), axis=AX.X, op=Alu.max,
                apply_absolute_value=True,
            )
            nc.vector.tensor_reduce(
                m2[:], half_view(x, 1), axis=AX.X, op=Alu.max,
                apply_absolute_value=True,
            )
            i_mcomb = nc.vector.tensor_tensor(m[:], m1[:], m2[:], op=Alu.max)

            # critical chain: m -> transpose -> bf16 -> 4 block-diag matmuls
            psT1 = ps.tile([R, B], f32)
            nc.tensor.transpose(psT1[:], m[:], ident[:])
            nc.scalar.copy(mT[:], psT1[:])
            tmpA = ps.tile([B, 2, 4 * OC], f32)
            tmpB = ps.tile([B, 2, 4 * OC], f32)
            for k in range(4):
                dst = tmpA if k < 2 else tmpB
                nc.tensor.matmul(
                    dst[:, k % 2, :], mT[:], d[:, k * 4 * OC : (k + 1) * 4 * OC]
                )

            # rest of DVE work behind the PE chain; s2 must not be scheduled
            # ahead of the chain-critical m combine (seen in the v5 trace)
            i_s1 = nc.vector.tensor_reduce(
                s1[:], half_view(x, 0), axis=AX.X, op=Alu.add,
                apply_absolute_value=True,
            )
            i_s2 = nc.vector.tensor_reduce(
                s2[:], half_view(x, 1), axis=AX.X, op=Alu.add,
                apply_absolute_value=True,
            )
            import concourse.tile as tile_mod

            tile_mod.add_dep_helper(
                i_s2.ins, i_mcomb.ins, sync=False, reason="keep m-chain first"
            )
            nc.vector.tensor_add(stack3[:, 2 * R : 3 * R], s1[:], s2[:])

            # A/B' pair-trees (contiguous halving of g), one per half
            def ab_tree(rs, tag):
                a1 = sb.tile([B, H], f32, tag=f"a1{tag}")
                a2 = sb.tile([B, H // 2], f32, tag=f"a2{tag}")
                a3 = sb.tile([B, H // 4], f32, tag=f"a3{tag}")
                ab = sb.tile([B, 2 * R], f32, tag=f"ab{tag}")
                for src, dst, w in (
                    (rs, a1, H), (a1, a2, H // 2), (a2, a3, H // 4),
                    (a3, ab, H // 8),
                ):
                    v = src[:].rearrange("p (w f) -> p w f", w=2)
                    nc.vector.tensor_tensor(
                        dst[:].rearrange("p (w f) -> p w f", w=2),
                        v[:, :, 0 : w // 2],
                        v[:, :, w // 2 : w],
                        op=Alu.add,
                    )
                return ab

            ab1 = ab_tree(rs1, "h1")
            ab2 = ab_tree(rs2, "h2")
            nc.vector.tensor_add(stack3[:, 0 : 2 * R], ab1[:], ab2[:])
            nc.gpsimd.memset(stack3[:, 3 * R : 3 * R + 1], 1.0)

            psT2 = ps.tile([3 * R + 1, B], f32)
            nc.tensor.transpose(psT2[:], stack3[:], ident[:])
            nc.scalar.copy(lhsT[:], psT2[:])
            pmm = ps.tile([B, OC], f32)
            nc.tensor.matmul(pmm[:], lhsT[:], rhs[:])

            # max over r: two strided reduces (start after 2 of 4 banks),
            # then combine + add the matmul part
            mpa = sb.tile([B, OC], f32)
            mpb = sb.tile([B, OC], f32)
            maxp = sb.tile([B, OC], f32)
            trA = tmpA[:].rearrange("p k (rr o) -> p o k rr", rr=4, o=OC)
            trB = tmpB[:].rearrange("p k (rr o) -> p o k rr", rr=4, o=OC)
            nc.vector.tensor_reduce(mpa[:], trA, axis=AX.XY, op=Alu.max)
            nc.vector.tensor_reduce(mpb[:], trB, axis=AX.XY, op=Alu.max)
            nc.vector.tensor_tensor(maxp[:], mpa[:], mpb[:], op=Alu.max)

            out_sb = sb.tile([B, OC], f32)
            nc.vector.tensor_add(out_sb[:], pmm[:], maxp[:])
            nc.sync.dma_start(out_d[:], out_sb[:])

    nc.compile()
    return nc


def _get_program():
    global _PROG
    if _PROG is None:
        _PROG = _build_program()
    return _PROG


def _host_inputs(x, weights):
    import ml_dtypes

    x = np.ascontiguousarray(np.asarray(x, dtype=np.float32))
    w = np.asarray(weights, dtype=np.float32)
    w16 = w[:R]  # (16, 1024) - only rows 0..15 are used by ROW_IDX
    in_maps = []
    for c in range(NCORES):
        wc = np.ascontiguousarray(w16[:, c * OC : (c + 1) * OC])  # (16,128)
        awc = np.abs(wc)
        d = np.zeros((R, R * OC), dtype=np.float32)
        for r in range(R):
            d[r, r * OC : (r + 1) * OC] = 0.1 * awc[r]
        rhs = np.concatenate(
            [wc, -0.5 * wc, -0.1 * awc, (-(G / 2.0) * wc.sum(axis=0))[None, :]],
            axis=0,
        ).astype(np.float32)  # (49, 128); mask count B = (B'+G)/2; bias row last
        in_maps.append(
            {
                "x": x,
                "d": d.astype(ml_dtypes.bfloat16),
                "rhs": np.ascontiguousarray(rhs),
            }
        )
    return in_maps


def kernel(x, weights):
    from concourse.bass_utils import run_bass_kernel_spmd

    nc = _get_program()
    in_maps = _host_inputs(x, weights)
    res = run_bass_kernel_spmd(nc, in_maps, core_ids=list(range(NCORES)))
    out = np.concatenate(
        [np.asarray(res.results[c]["out"]) for c in range(NCORES)], axis=1
    )
    return out.astype(np.float32)

